# revision 38
# baseline (speedup 1.0000x reference)
"""Multi-head attention Trainium2 Bass kernel (8 NeuronCores).

Problem: B=2, S=2048, D=1024, H=16, Dh=64, scale=1/sqrt(D).
Sharding: batch x head. Core c handles batch c//4, heads (c%4)*4 .. +4.
No collectives: per-core partial outputs are combined on the host
(softmax normalization + head sum + b_o add), which is O(B*H*S*Dh) work.

Per-core pipeline (v2 — fp8 DoubleRow for the q/k path):
  1. q/k projection in fp8e4m3 DoubleRow perf mode (0.5 cycles/row):
     x and W_q/W_k are host-prequantized to fp8 (W scaled by 32 to stay
     in the normal range; the 32*32=1024 factor plus the 1/sqrt(D)
     softmax scale are folded into the exp activation's scale).
     Output q8/k8 stored as fp8 [128, 2, S]: partitions 32h..32h+31 hold
     head h dims d%32, slice dim = d//32 — the DoubleRow operand layout
     for the logits matmul.  v projection in bf16 (v feeds the output
     directly, fp8 would cost too much precision).
  2. Attention per (q-block of 512, head-pair g):
     logitsT [sk,sq] per head via one fp8 DoubleRow matmul per
     (sk-chunk of 128): lhsT=k8 [32,2,128], rhs=q8 [32,2,512].
     exp on ScalarE with scale=1/(32*32*sqrt(D)) (PSUM -> bf16 SBUF),
     multiplicative {0,1} bf16 mask on VectorE (2x mode), then
     ctxT [65, 512] accumulated as (v|1).T @ P in bf16 — row 64 =
     softmax denominators.  ctx+proj of head-pair g-1 are emitted after
     the logits of g so the PE stays busy while ACT works through exp.
  3. Per-head output projection Wo_h.T @ cu on device; DMA psum->HBM.
Host combine: divide by denominators, sum heads, add b_o.
"""

import numpy as np
import ml_dtypes

import concourse.bass as bass  # noqa: F401
import concourse.tile as tile
from concourse import bacc, mybir
from concourse.bass_utils import run_bass_kernel_spmd

B, S, D = 2, 2048, 1024
H, Dh = 16, 64
NCORE = 8
GPB = NCORE // B            # cores per batch (4)
HL = H // GPB               # local heads per core (4)
SCALE = float(1.0 / np.sqrt(np.float32(D)))
WSCL = 32.0                 # fp8 W_q/W_k pre-scale (keeps values normal)
EXPSCL = SCALE / (WSCL * WSCL)

F32 = mybir.dt.float32
F32R = mybir.dt.float32r
BF16 = mybir.dt.bfloat16
F8 = mybir.dt.float8e4
DR = mybir.MatmulPerfMode.DoubleRow

NP_F8 = mybir.dt.np(F8)

NCH = S // 128    # 16 sk chunks
NQG = S // 512    # 4 query groups
NPAIR = D // 256  # 4 DoubleRow contraction pairs
USE_POOL_MASK = False   # real-HW gpsimd elementwise is ~10x the cost model


def build_module(reps=1, zero_bias=False):
    nc = bacc.Bacc("TRN2", target_bir_lowering=False, debug=False,
                   num_devices=NCORE)

    x8d = nc.dram_tensor("x8", [NQG, 128, NPAIR * 2 * 512], F8,
                         kind="ExternalInput").ap()
    xvd = nc.dram_tensor("xv", [D // 128, 128, S], BF16, kind="ExternalInput").ap()
    wqk8d = nc.dram_tensor("wqk8", [128, NPAIR * 2 * 512], F8,
                           kind="ExternalInput").ap()
    wvd = nc.dram_tensor("wv", [D // 128, 128, HL * Dh], BF16,
                         kind="ExternalInput").ap()
    bqkd = nc.dram_tensor("bqk", [128, 4], F32, kind="ExternalInput").ap()
    bvd = nc.dram_tensor("bv", [128, HL * Dh], F32, kind="ExternalInput").ap()
    wod = nc.dram_tensor("wo", [Dh, HL * Dh], F32, kind="ExternalInput").ap()
    maskTd = nc.dram_tensor("maskT", [S, S], BF16, kind="ExternalInput").ap()
    outp = nc.dram_tensor("outp", [HL, Dh, S], F32,
                          kind="ExternalOutput").ap()
    ssum = nc.dram_tensor("ssum", [HL, S], F32,
                          kind="ExternalOutput").ap()

    with tile.TileContext(nc) as tc:
        # ---------------- persistent tiles ----------------
        with (
            tc.tile_pool(name="const", bufs=1) as constp,
            tc.tile_pool(name="qk", bufs=1) as qkp,
            tc.tile_pool(name="vpool", bufs=1) as vpoolp,
        ):
            wqk8_sb = constp.tile([128, NPAIR * 2 * 512], F8)
            nc.sync.dma_start(wqk8_sb, wqk8d)
            wv_sb = constp.tile([128, (D // 128) * HL * Dh], BF16)
            nc.sync.dma_start(
                wv_sb.rearrange("p (d f) -> p d f", d=D // 128),
                wvd.rearrange("d p f -> p d f"))
            bqk_sb = constp.tile([128, 4], F32)
            nc.sync.dma_start(bqk_sb, bqkd)
            bv_sb = constp.tile([128, HL * Dh], F32)
            nc.sync.dma_start(bv_sb, bvd)
            wo_sb = constp.tile([Dh, HL * Dh], F32R)
            with tc.tile_pool(name="wstage", bufs=1, side="right") as wstp:
                wo_st = wstp.tile([Dh, HL * Dh], F32)
                nc.sync.dma_start(wo_st, wod)
                nc.vector.tensor_copy(wo_sb, wo_st)
            wo_r = wo_sb

            # q8/k8 per head-pair tile t: partitions 32*(h%2)..+31 = head
            # 2t+(h%2), free = (slice d//32, seq).  Two tiles per tensor so
            # matmul base partitions stay in {0, 32} (96 is not encodable).
            q8 = [qkp.tile([64, 2 * S], F8, name=f"q8{t}") for t in range(2)]
            k8 = [qkp.tile([64, 2 * S], F8, name=f"k8{t}") for t in range(2)]
            q8v = [t.rearrange("p (two s) -> p two s", two=2) for t in q8]
            k8v = [t.rearrange("p (two s) -> p two s", two=2) for t in k8]
            # v in [sk, d] layout: per sk-chunk j, per head h: 64 cols + one
            v_sb = vpoolp.tile([128, NCH * HL * 65], BF16)
            nc.vector.memset(
                v_sb.rearrange("p (m c) -> p m c", c=65)[:, :, 64:65], 1.0)

        for _rep in range(reps):
            # Single scope: phase A (k projection for all of S, q for qg=0)
            # flows directly into phase B (attention).  q for qg>0 and the
            # whole v projection are emitted inside phase B where the PE has
            # slack while ACT works through exp.  PSUM: ptps 4 banks,
            # scratch (qk/v/po) 2 banks, ctx 2 banks.
            with (
                tc.tile_pool(name="xtp", bufs=4, side="right") as xtp,
                tc.tile_pool(name="xvp", bufs=2, side="right") as xvp,
                tc.tile_pool(name="maskp", bufs=2, side="right") as maskp,
                tc.tile_pool(name="ptp", bufs=2, side="right") as ptp,
                tc.tile_pool(name="ptps", space="PSUM", bufs=2) as ptpsp,
                tc.tile_pool(name="scps", space="PSUM", bufs=2) as scpsp,
                tc.tile_pool(name="ctxps", space="PSUM", bufs=2) as ctxpsp,
                tc.tile_pool(name="cup", bufs=3, side="right") as cup,
            ):
                wqk8v = wqk8_sb.rearrange("p (c two f) -> p c two f",
                                          c=NPAIR, two=2)
                x8ts = [None] * NQG

                def dma_x8(sb4, split=False):
                    x8t = xtp.tile([128, NPAIR * 2 * 512], F8,
                                   tag=f"x8{sb4}", bufs=1)
                    if split:
                        for c in range(NPAIR):
                            nc.sync.dma_start(
                                x8t[:, c * 1024:(c + 1) * 1024],
                                x8d[sb4][:, c * 1024:(c + 1) * 1024])
                    else:
                        nc.sync.dma_start(x8t, x8d[sb4])
                    x8ts[sb4] = x8t.rearrange("p (c two s) -> p c two s",
                                              c=NPAIR, two=2)

                def emit_qk_block(sb4, blk, dve_only=False):
                    # blk: 0=q-lo, 1=q-hi, 2=k-lo, 3=k-hi (4h x 32 cols)
                    ps = scpsp.tile([128, 512], F32, tag="sc", bufs=2)
                    for c in range(NPAIR):
                        nc.tensor.matmul(
                            ps,
                            lhsT=wqk8v[:, c, :, blk * 128:(blk + 1) * 128],
                            rhs=x8ts[sb4][:, c, :, :],
                            start=(c == 0), stop=(c == NPAIR - 1),
                            perf_mode=DR)
                    tgts = q8v if blk < 2 else k8v
                    nc.vector.tensor_scalar_add(
                        tgts[0][:, blk % 2, sb4 * 512:(sb4 + 1) * 512],
                        ps[0:64, :], bqk_sb[0:64, blk:blk + 1])
                    if dve_only:
                        nc.vector.tensor_scalar_add(
                            tgts[1][:, blk % 2, sb4 * 512:(sb4 + 1) * 512],
                            ps[64:128, :], bqk_sb[64:128, blk:blk + 1])
                    else:
                        nc.scalar.activation(
                            tgts[1][:, blk % 2, sb4 * 512:(sb4 + 1) * 512],
                            ps[64:128, :],
                            mybir.ActivationFunctionType.Identity,
                            bias=bqk_sb[64:128, blk:blk + 1])

                # minimal phase A: x8(0), k(0), q(0); k(1..3) trickle into
                # unit (0,0) per-j
                dma_x8(0)
                emit_qk_block(0, 2)
                emit_qk_block(0, 3)
                emit_qk_block(0, 0)
                emit_qk_block(0, 1)

                # v-projection thunks, interleaved into unit (0, 1)
                xvts = [None]

                def dma_xv(sb4):
                    xvt = xvp.tile([128, (D // 128) * 512], BF16,
                                   tag="xv", bufs=2)
                    nc.sync.dma_start(
                        xvt.rearrange("p (d s) -> p d s", d=D // 128),
                        xvd.rearrange("d p s -> p d s")
                           [:, :, sb4 * 512:(sb4 + 1) * 512])
                    return xvt.rearrange("p (d s) -> p d s", d=D // 128)

                def v_chunk(j):
                    sb4, jj = j // 4, j % 4
                    if jj == 0:
                        xvts[0] = dma_xv(sb4)
                    xvtv = xvts[0]
                    psv = scpsp.tile([128, HL * Dh], F32, tag="sc", bufs=2)
                    for d in range(D // 128):
                        nc.tensor.matmul(
                            psv,
                            lhsT=xvtv[:, d, jj * 128:(jj + 1) * 128],
                            rhs=wv_sb[:, d * 256:(d + 1) * 256],
                            start=(d == 0), stop=(d == D // 128 - 1))
                    vtgt = (v_sb[:, j * (HL * 65):(j + 1) * (HL * 65)]
                            .rearrange("p (h c) -> p h c", h=HL)[:, :, 0:64])
                    if False:
                        # b_v == 0: plain copy, on the less-loaded ACT
                        nc.scalar.activation(
                            vtgt, psv.rearrange("p (h c) -> p h c", h=HL),
                            mybir.ActivationFunctionType.Copy)
                    else:
                        nc.vector.tensor_add(
                            vtgt,
                            psv.rearrange("p (h c) -> p h c", h=HL),
                            bv_sb.rearrange("p (h c) -> p h c", h=HL))

                def emit_ctx_head(qg, g, hh, ptv, js, ctx):
                    for j in js:
                        h = 2 * g + hh
                        nc.tensor.matmul(
                            ctx,
                            lhsT=v_sb[:, j * (HL * 65) + h * 65:
                                      j * (HL * 65) + (h + 1) * 65],
                            rhs=ptv[:, j, hh, :],
                            start=(j == 0), stop=(j == NCH - 1))

                def emit_out(qg, g, hh, ctx, use_act=False):
                    h = 2 * g + hh
                    cu = cup.tile([65, 512], F32R, tag="cu", bufs=3)
                    nc.vector.tensor_copy(cu, ctx)
                    nc.sync.dma_start(
                        ssum[h:h + 1, qg * 512:(qg + 1) * 512],
                        cu[64:65, :].bitcast(F32))
                    po = scpsp.tile([64, 512], F32, tag="sc", bufs=2)
                    nc.tensor.matmul(
                        po,
                        lhsT=wo_r[:, h * 64:(h + 1) * 64],
                        rhs=cu[0:64, :],
                        start=True, stop=True)
                    po_sb = cup.tile([64, 512], F32, tag="po_sb", bufs=3)
                    if use_act:
                        nc.scalar.activation(
                            po_sb, po, mybir.ActivationFunctionType.Copy)
                    else:
                        nc.vector.tensor_copy(po_sb, po)
                    nc.sync.dma_start(
                        outp[h][:, qg * 512:(qg + 1) * 512],
                        po_sb)

                def emit_ctx_proj(qg, g, ptt):
                    ptv = ptt.rearrange("p (j e c) -> p j e c", j=NCH, e=2)
                    for hh in range(2):
                        ctx = ctxpsp.tile([65, 512], F32, tag="ctx", bufs=2)
                        emit_ctx_head(qg, g, hh, ptv, range(NCH), ctx)
                        emit_out(qg, g, hh, ctx)

                prev = None  # (qg, g, ptv, [ctx_hh0, ctx_hh1])
                for qg in range(NQG):
                    mts = []
                    for r in range(2):
                        mt = maskp.tile([128, 8 * 512], BF16,
                                        name=f"mt{r}", tag=f"mask{r}", bufs=2)
                        nc.sync.dma_start(
                            mt.rearrange("p (j c) -> p j c", j=8),
                            maskTd.rearrange("(j p) q -> p j q", p=128)
                                  [:, 8 * r:8 * r + 8,
                                   qg * 512:(qg + 1) * 512])
                        mts.append(mt.rearrange("p (j c) -> p j c", j=8))
                    for g in range(HL // 2):
                        last = (qg == NQG - 1 and g == HL // 2 - 1)
                        if g == 0 and qg > 0:
                            # q projection for this qg (PE slack)
                            emit_qk_block(qg, 0, dve_only=True)
                            emit_qk_block(qg, 1, dve_only=True)
                        ptt = ptp.tile([128, NCH * 1024], BF16, tag="pt",
                                       bufs=2)
                        ptv = ptt.rearrange("p (j e c) -> p j e c",
                                            j=NCH, e=2)
                        lctx = None
                        for j in range(NCH):
                            pps = ptpsp.tile([128, 1024], F32, tag="ptps",
                                             bufs=2)
                            for hh in range(2):
                                h = 2 * g + hh
                                t, p0 = h // 2, 32 * (h % 2)
                                nc.tensor.matmul(
                                    pps[:, hh * 512:(hh + 1) * 512],
                                    lhsT=k8v[t][p0:p0 + 32, :,
                                                j * 128:(j + 1) * 128],
                                    rhs=q8v[t][p0:p0 + 32, :,
                                               qg * 512:(qg + 1) * 512],
                                    start=True, stop=True,
                                    perf_mode=DR)
                            nc.scalar.activation(
                                ptt[:, j * 1024:(j + 1) * 1024], pps,
                                mybir.ActivationFunctionType.Exp,
                                scale=EXPSCL)
                            if qg == 0 and g == 0:
                                if j < 6:
                                    # trickle k(1..3): dma then blocks
                                    sb4 = j // 2 + 1
                                    if j % 2 == 0:
                                        dma_x8(sb4)
                                        emit_qk_block(sb4, 2, dve_only=True)
                                    else:
                                        emit_qk_block(sb4, 3, dve_only=True)
                                else:
                                    v_chunk(j - 6)      # chunks 0..9
                            if qg == 0 and g == 1 and j < 6:
                                v_chunk(j + 10)         # chunks 10..15
                            if j % 2 == 1:
                                jp = j // 2
                                r, jr = jp // 4, 2 * jp % 8
                                for e in range(2):
                                    eng = (nc.gpsimd if (USE_POOL_MASK and qg == 0 and e == 1)
                                           else nc.vector)
                                    eng.tensor_mul(
                                        ptv[:, j - 1:j + 1, e, :],
                                        ptv[:, j - 1:j + 1, e, :],
                                        mts[r][:, jr:jr + 2, :])
                            # drain previous unit's ctx: 2 chunks per j.
                            # At unit (0,1) the drain shifts to j>=8 so the
                            # interleaved v projection (chunk j emitted this
                            # same iteration) stays ahead of the reads.
                            doff = 6 if (qg == 0 and g == 1) else 0
                            if prev is not None:
                                pqg, pg, pptv, pctx = prev
                                if doff <= j < doff + 8:
                                    p = j - doff
                                    for hh in range(2):
                                        emit_ctx_head(pqg, pg, hh, pptv,
                                                      (2 * p, 2 * p + 1),
                                                      pctx[hh])
                                elif j == doff + 8:
                                    for hh in range(2):
                                        emit_out(pqg, pg, hh, pctx[hh])
                                    prev = None
                            if last and j % 2 == 1 and j >= 9:
                                # own-ctx catch-up: chunks emitted only after
                                # prev's banks are freed (outs at j==8)
                                sched = {9: range(0, 6), 11: range(6, 10),
                                         13: range(10, 14), 15: range(14, 16)}
                                if lctx is None:
                                    lctx = [ctxpsp.tile(
                                        [65, 512], F32, tag="ctx",
                                        name=f"lctx{i}", bufs=2)
                                        for i in range(2)]
                                for hh in range(2):
                                    emit_ctx_head(qg, g, hh, ptv,
                                                  sched[j], lctx[hh])
                        if last:
                            for hh in range(2):
                                emit_out(qg, g, hh, lctx[hh], use_act=True)
                        elif prev is not None:
                            # shouldn't happen (drained at j==8), but be safe
                            pqg, pg, pptv, pctx = prev
                            for hh in range(2):
                                emit_out(pqg, pg, hh, pctx[hh])
                            prev = None
                        if not last:
                            ctxs = [ctxpsp.tile([65, 512], F32, tag="ctx",
                                                name=f"ctx{i}", bufs=2)
                                    for i in range(2)]
                            prev = (qg, g, ptv, ctxs)

    nc.compile()
    return nc


_NC_CACHE = {}


def get_module(reps=1, zero_bias=False):
    key = (reps, zero_bias)
    if key not in _NC_CACHE:
        _NC_CACHE[key] = build_module(reps, zero_bias=zero_bias)
    return _NC_CACHE[key]


def make_in_maps(x, W_qkv, b_qkv, W_o, b_o, mask):
    x = np.asarray(x, np.float32)
    W_qkv = np.asarray(W_qkv, np.float32)
    b_qkv = np.asarray(b_qkv, np.float32)
    W_o = np.asarray(W_o, np.float32)
    mask = np.asarray(mask)

    # reference layout: W_qkv[:, h*3*Dh + {0..Dh | Dh..2Dh | 2Dh..3Dh}] =
    # q|k|v of head h
    W3 = W_qkv.reshape(D, H, 3 * Dh)
    b3 = b_qkv.reshape(H, 3 * Dh)
    Wq = np.ascontiguousarray(W3[:, :, :Dh].reshape(D, H * Dh))
    Wk = np.ascontiguousarray(W3[:, :, Dh:2 * Dh].reshape(D, H * Dh))
    Wv = np.ascontiguousarray(W3[:, :, 2 * Dh:].reshape(D, H * Dh))
    bq = np.ascontiguousarray(b3[:, :Dh].reshape(H * Dh))
    bk = np.ascontiguousarray(b3[:, Dh:2 * Dh].reshape(H * Dh))
    bv_full = np.ascontiguousarray(b3[:, 2 * Dh:].reshape(H * Dh))

    xT_b = []
    for b in range(B):
        xT = np.ascontiguousarray(x[b].T)                        # [D, S]
        # x8[sb4, p, (c, s, q)] = xT[256c + 128s + p, 512*sb4 + q]
        x8 = np.ascontiguousarray(
            xT.reshape(NPAIR, 2, 128, NQG, 512).transpose(3, 2, 0, 1, 4)
            .reshape(NQG, 128, NPAIR * 2 * 512)
        ).astype(NP_F8)
        xv = xT.reshape(D // 128, 128, S).astype(ml_dtypes.bfloat16)
        xT_b.append((x8, xv))
    maskT_b = [np.ascontiguousarray(
        (mask[b, 0] != 0).T.astype(ml_dtypes.bfloat16)) for b in range(B)]

    in_maps = []
    for c in range(NCORE):
        b = c // GPB
        g0 = (c % GPB) * HL  # first global head of this core
        # wqk8 blocks: [q-lo, q-hi, k-lo, k-hi], each 4 heads x 32 cols
        blocks = []
        for (Wm, lo) in ((Wq, 0), (Wq, 32), (Wk, 0), (Wk, 32)):
            cols = [Wm[:, (g0 + h) * 64 + lo:(g0 + h) * 64 + lo + 32]
                    for h in range(HL)]
            blocks.append(np.concatenate(cols, axis=1))          # [D, 128]
        Wblk = np.concatenate(blocks, axis=1) * WSCL             # [D, 512]
        # wqk8[p, (c, s, f)] = Wblk[256c + 128s + p, f]
        wqk8 = np.ascontiguousarray(
            Wblk.reshape(NPAIR, 2, 128, 512).transpose(2, 0, 1, 3)
            .reshape(128, NPAIR * 2 * 512)
        ).astype(NP_F8)

        wv_c = np.ascontiguousarray(
            Wv[:, g0 * 64:(g0 + HL) * 64].reshape(D // 128, 128, HL * Dh)
        ).astype(ml_dtypes.bfloat16)                             # [8,128,256]

        bqk_c = np.zeros((128, 4), np.float32)
        for blk, (bm, lo) in enumerate(((bq, 0), (bq, 32), (bk, 0), (bk, 32))):
            for p in range(128):
                h, d = p // 32, p % 32
                bqk_c[p, blk] = bm[(g0 + h) * 64 + lo + d] * WSCL

        bv_c = np.tile(bv_full[g0 * 64:(g0 + HL) * 64], (128, 1))
        wo_c = np.concatenate(
            [W_o[(g0 + h) * 64:(g0 + h + 1) * 64, :] for h in range(HL)],
            axis=1)
        in_maps.append({
            "x8": xT_b[b][0],
            "xv": xT_b[b][1],
            "wqk8": wqk8,
            "wv": wv_c,
            "bqk": np.ascontiguousarray(bqk_c, dtype=np.float32),
            "bv": np.ascontiguousarray(bv_c, dtype=np.float32),
            "wo": np.ascontiguousarray(wo_c, dtype=np.float32),
            "maskT": maskT_b[b],
        })
    return in_maps


def combine_outputs(results, b_o):
    """results: list of 8 dicts with 'outp' [HL, Dh, S] and 'ssum' [HL, S]."""
    b_o = np.asarray(b_o, np.float32)
    out = np.zeros((B, S, Dh), np.float32)
    for c in range(NCORE):
        b = c // GPB
        op = results[c]["outp"].astype(np.float32)    # [HL, Dh, S]
        ss = results[c]["ssum"].astype(np.float32)    # [HL, S]
        contrib = (op / ss[:, None, :]).sum(axis=0)   # [Dh, S]
        out[b] += contrib.T
    out += b_o[None, None, :]
    return out


def kernel(x, W_qkv, b_qkv, W_o, b_o, mask):
    nc = get_module()
    in_maps = make_in_maps(x, W_qkv, b_qkv, W_o, b_o, mask)
    res = run_bass_kernel_spmd(nc, in_maps, core_ids=list(range(NCORE)))
    return combine_outputs(res.results, b_o)
